# revision 7
# baseline (speedup 1.0000x reference)
"""AttentionHead kernel for 8 TRN2 NeuronCores (Bass/Tile).

Problem: x[4, 2048, 1024] f32; Wq/Wk/Wv[1024, 1024], bq/bk/bv[1024].
  q = x@Wq+bq ; k = x@Wk+bk ; v = x@Wv+bv
  out = softmax(q k^T / sqrt(1024)) @ v

Sharding: 8 shards = (batch b in 0..3) x (query-half h in 0..1).
Core c = 2*b + h computes output rows [h*1024, (h+1)*1024) of batch b.
Each core does its own K/V projection for the full 2048-token sequence
(duplicated across the pair of cores sharing a batch).

Compute dtype: bf16 operands with f32 PSUM accumulation.
"""

import numpy as np
import ml_dtypes

B = 4
S = 2048
D = 1024
HALF = S // 2  # query rows per core
NCORES = 8
DCH = D // 128  # 8 contraction chunks
BF = ml_dtypes.bfloat16

_cache = {}


def _build(use_bias: bool):
    import concourse.bass as bass
    import concourse.mybir as mybir
    import concourse.tile as tile
    from concourse import bacc
    from concourse.masks import make_identity

    FP32 = mybir.dt.float32
    BF16 = mybir.dt.bfloat16
    AF = mybir.ActivationFunctionType

    nc = bacc.Bacc(
        "TRN2",
        target_bir_lowering=False,
        debug=False,
        enable_asserts=True,
        num_devices=NCORES,
    )

    # Per-core inputs.
    xkv_d = nc.dram_tensor("xkv", [S, D], BF16, kind="ExternalInput").ap()
    xq_d = nc.dram_tensor("xq", [HALF, D], BF16, kind="ExternalInput").ap()
    # Weights in e-chunk-major layout [e_chunk, d, 128] (Wq pre-scaled by 1/32).
    wq_d = nc.dram_tensor("wq", [DCH, D, 128], BF16, kind="ExternalInput").ap()
    wk_d = nc.dram_tensor("wk", [DCH, D, 128], BF16, kind="ExternalInput").ap()
    # Wv natural [d, e] layout.
    wv_d = nc.dram_tensor("wv", [D, D], BF16, kind="ExternalInput").ap()
    if use_bias:
        # bq pre-scaled by 1/32; layout [e_chunk, 128].
        bq_d = nc.dram_tensor("bq", [DCH, 128], mybir.dt.float32,
                              kind="ExternalInput").ap()
        bk_d = nc.dram_tensor("bk", [DCH, 128], mybir.dt.float32,
                              kind="ExternalInput").ap()
        bv_d = nc.dram_tensor("bv", [1, D], mybir.dt.float32,
                              kind="ExternalInput").ap()
    out_d = nc.dram_tensor("out", [HALF, D], FP32, kind="ExternalOutput").ap()

    with tile.TileContext(nc) as tc:
        with (
            tc.tile_pool(name="persist", bufs=1) as persist,
            tc.tile_pool(name="wstream", bufs=2) as wpool,
            tc.tile_pool(name="ppool", bufs=2) as ppool,
            tc.tile_pool(name="stat", bufs=2) as statpool,
            tc.tile_pool(name="opool", bufs=2) as opool,
            tc.tile_pool(name="psA", bufs=1, space="PSUM") as psA,
            tc.tile_pool(name="psB", bufs=2, space="PSUM") as psB,
            tc.tile_pool(name="psO", bufs=1, space="PSUM") as psO,
        ):
            # ---- persistent tiles ----
            ident = persist.tile([128, 128], BF16, tag="ident", name="ident")
            make_identity(nc, ident)

            xt = [persist.tile([128, S], BF16, tag=f"xt{d}", name=f"xt{d}")
                  for d in range(DCH)]
            xqt = [persist.tile([128, HALF], BF16, tag=f"xqt{d}", name=f"xqt{d}")
                   for d in range(DCH)]
            wv_sb = [persist.tile([128, D], BF16, tag=f"wv{d}", name=f"wv{d}")
                     for d in range(DCH)]
            kT = [persist.tile([128, S], BF16, tag=f"kT{m}", name=f"kT{m}")
                  for m in range(DCH)]
            qT = [persist.tile([128, HALF], BF16, tag=f"qT{m}", name=f"qT{m}")
                  for m in range(DCH)]
            v_sb = [persist.tile([128, D], BF16, tag=f"v{t}", name=f"v{t}")
                    for t in range(S // 128)]

            if use_bias:
                bq_sb = persist.tile([128, DCH], mybir.dt.float32, tag="bq",
                                     name="bq_sb")
                bk_sb = persist.tile([128, DCH], mybir.dt.float32, tag="bk",
                                     name="bk_sb")
                bv_row = persist.tile([1, D], mybir.dt.float32, tag="bvr",
                                      name="bv_row")
                bv_bc = persist.tile([128, D], mybir.dt.float32, tag="bvb",
                                     name="bv_bc")
                nc.sync.dma_start(bq_sb, bq_d.rearrange("a b -> b a"))
                nc.sync.dma_start(bk_sb, bk_d.rearrange("a b -> b a"))
                nc.sync.dma_start(bv_row, bv_d)
                nc.gpsimd.partition_broadcast(bv_bc, bv_row)

            # ---- load x (DMA transpose on sync ring; weights go on the
            # scalar HWDGE ring so they don't queue behind the transposes) ----
            for d in range(DCH):
                nc.sync.dma_start_transpose(xqt[d], xq_d[:, d * 128:(d + 1) * 128])
            for d in range(DCH):
                nc.sync.dma_start_transpose(xt[d], xkv_d[:, d * 128:(d + 1) * 128])
            for d in range(DCH):
                nc.scalar.dma_start(wv_sb[d], wv_d[d * 128:(d + 1) * 128, :])

            # ---- Q^T projection: qT[m][e, q] = sum_d Wq[d, e] * xq[q, d] ----
            for m in range(DCH):
                wq_sb = wpool.tile([128, DCH, 128], BF16, tag="wq", name="wq_sb")
                nc.scalar.dma_start(wq_sb, wq_d[m].rearrange("(jd p) e -> p jd e",
                                                             p=128))
                for qf in range(HALF // 512):
                    ps = psB.tile([128, 512], FP32, tag="ps_small", name="ps_q")
                    for jd in range(DCH):
                        nc.tensor.matmul(
                            ps,
                            wq_sb[:, jd, :],
                            xqt[jd][:, qf * 512:(qf + 1) * 512],
                            start=(jd == 0),
                            stop=(jd == DCH - 1),
                        )
                    dst = qT[m][:, qf * 512:(qf + 1) * 512]
                    if use_bias:
                        nc.scalar.activation(dst, ps, AF.Identity,
                                             bias=bq_sb[:, m:m + 1])
                    else:
                        nc.scalar.copy(dst, ps)

            # ---- K^T projection: kT[m][e, t] over full sequence ----
            for m in range(DCH):
                wk_sb = wpool.tile([128, DCH, 128], BF16, tag="wk", name="wk_sb")
                nc.scalar.dma_start(wk_sb, wk_d[m].rearrange("(jd p) e -> p jd e",
                                                             p=128))
                for tf in range(S // 512):
                    ps = psB.tile([128, 512], FP32, tag="ps_small", name="ps_k")
                    for jd in range(DCH):
                        nc.tensor.matmul(
                            ps,
                            wk_sb[:, jd, :],
                            xt[jd][:, tf * 512:(tf + 1) * 512],
                            start=(jd == 0),
                            stop=(jd == DCH - 1),
                        )
                    dst = kT[m][:, tf * 512:(tf + 1) * 512]
                    if use_bias:
                        nc.scalar.activation(dst, ps, AF.Identity,
                                             bias=bk_sb[:, m:m + 1])
                    else:
                        nc.scalar.copy(dst, ps)

            # ---- V projection: v[t][token, e] = sum_d x[token, d] * Wv[d, e] ----
            # (v bias is deferred to the output: sum_j w_ij = 1.)
            for t in range(S // 128):
                for ef in range(D // 512):
                    ps = psB.tile([128, 512], FP32, tag="ps_small", name="ps_v")
                    for jd in range(DCH):
                        nc.tensor.matmul(
                            ps,
                            xt[jd][:, t * 128:(t + 1) * 128],
                            wv_sb[jd][:, ef * 512:(ef + 1) * 512],
                            start=(jd == 0),
                            stop=(jd == DCH - 1),
                        )
                    nc.scalar.copy(v_sb[t][:, ef * 512:(ef + 1) * 512], ps)

            # ---- attention, one 128-row query block at a time ----
            # Software-pipelined emission: block qb's P-transposes + PV
            # matmuls are emitted after block qb+1's score matmuls, so the
            # PE fills the reduce_max+exp latency of block qb+1 with block
            # qb's tail work (also keeps HAM warm).
            NQB = HALF // 128
            pend = {}

            def emit_tail(qb, P, rinv):
                # transpose P 128x128 blocks: pT[t*128:, q] = P[q, t*128:]
                pT = ppool.tile([128, S], BF16, tag="pT", name="pT")
                for jj in range(S // 512):
                    psT = psB.tile([128, 512], BF16, tag="ps_small", name="ps_t")
                    for u in range(4):
                        j = jj * 4 + u
                        nc.tensor.transpose(psT[:, u * 128:(u + 1) * 128],
                                            P[:, j * 128:(j + 1) * 128], ident)
                    nc.vector.tensor_copy(pT[:, jj * 512:(jj + 1) * 512], psT)

                psout = psO.tile([128, D], FP32, tag="psout", name="psout")
                for ef in range(D // 512):
                    for j in range(S // 128):
                        nc.tensor.matmul(
                            psout[:, ef * 512:(ef + 1) * 512],
                            pT[:, j * 128:(j + 1) * 128],
                            v_sb[j][:, ef * 512:(ef + 1) * 512],
                            start=(j == 0),
                            stop=(j == S // 128 - 1),
                        )

                osb = opool.tile([128, D], FP32, tag="osb", name="osb")
                nc.vector.tensor_scalar_mul(osb, psout, rinv)
                if use_bias:
                    nc.vector.tensor_add(osb, osb, bv_bc)
                nc.sync.dma_start(out_d[qb * 128:(qb + 1) * 128, :], osb)

            for qb in range(NQB):
                psS = psA.tile([128, S], FP32, tag="psS", name="psS")
                for tf in range(S // 512):
                    for m in range(DCH):
                        nc.tensor.matmul(
                            psS[:, tf * 512:(tf + 1) * 512],
                            qT[m][:, qb * 128:(qb + 1) * 128],
                            kT[m][:, tf * 512:(tf + 1) * 512],
                            start=(m == 0),
                            stop=(m == DCH - 1),
                        )

                negmax = statpool.tile([128, 1], FP32, tag="negmax", name="negmax")
                nc.vector.reduce_max(negmax, psS, axis=mybir.AxisListType.X,
                                     negate=True)

                P = ppool.tile([128, S], BF16, tag="P", name="P")
                rowsum = statpool.tile([128, 1], FP32, tag="rowsum", name="rowsum")
                nc.scalar.activation(P, psS, AF.Exp, bias=negmax, scale=1.0,
                                     accum_out=rowsum)
                rinv = statpool.tile([128, 1], FP32, tag="rinv", name="rinv")
                nc.vector.reciprocal(rinv, rowsum)
                pend[qb] = (P, rinv)

                if qb > 0:
                    emit_tail(qb - 1, *pend.pop(qb - 1))
            emit_tail(NQB - 1, *pend.pop(NQB - 1))

    nc.compile()
    return nc


def _get_nc(use_bias: bool):
    key = ("nc", use_bias)
    if key not in _cache:
        _cache[key] = _build(use_bias)
    return _cache[key]


def _prep_inputs(x, Wq, bq, Wk, bk, Wv, bv, use_bias):
    """Host-side shard + layout prep. Returns in_maps for cores 0..7."""
    scale = 1.0 / np.sqrt(np.float32(D))

    # e-chunk-major weights: [e_chunk, d, 128]
    def echunk(w):
        return np.ascontiguousarray(
            w.reshape(D, DCH, 128).transpose(1, 0, 2)).astype(BF)

    wq_r = echunk(np.asarray(Wq, dtype=np.float32) * scale)
    wk_r = echunk(np.asarray(Wk, dtype=np.float32))
    wv_r = np.asarray(Wv, dtype=np.float32).astype(BF)

    if use_bias:
        bq_r = np.ascontiguousarray(
            (np.asarray(bq, np.float32) * scale).reshape(DCH, 128))
        bk_r = np.ascontiguousarray(np.asarray(bk, np.float32).reshape(DCH, 128))
        bv_r = np.ascontiguousarray(np.asarray(bv, np.float32).reshape(1, D))

    x = np.asarray(x, dtype=np.float32)
    in_maps = []
    for c in range(NCORES):
        b, h = c // 2, c % 2
        xb = np.ascontiguousarray(x[b]).astype(BF)
        xqh = np.ascontiguousarray(xb[h * HALF:(h + 1) * HALF, :])
        m = {"xkv": xb, "xq": xqh, "wq": wq_r, "wk": wk_r, "wv": wv_r}
        if use_bias:
            m.update({"bq": bq_r, "bk": bk_r, "bv": bv_r})
        in_maps.append(m)
    return in_maps


def _enable_jax_cache():
    try:
        import jax

        jax.config.update("jax_compilation_cache_dir", "/tmp/jax_neff_cache")
        jax.config.update("jax_persistent_cache_min_compile_time_secs", 0.0)
        jax.config.update("jax_persistent_cache_min_entry_size_bytes", -1)
    except Exception:
        pass


def _install_ntff_hook_shim():
    """The agent image's antenv lacks axon_hooks; synthesize it from
    trn_boot's ctypes implementation so trace=True can profile."""
    import sys
    import types

    if "antenv.axon_hooks" in sys.modules:
        return
    try:
        import antenv
        from trn_agent_boot.trn_boot import _ntff_profile_via_ctypes

        hook = _ntff_profile_via_ctypes("/opt/axon/libaxon_pjrt.so")
        mod = types.ModuleType("antenv.axon_hooks")
        state = {"h": hook}
        mod.get_axon_ntff_profile_hook = lambda: state["h"]
        mod.set_axon_ntff_profile_hook = lambda h: state.update(h=h)
        antenv.axon_hooks = mod
        sys.modules["antenv.axon_hooks"] = mod
    except Exception as e:
        print(f"ntff hook shim failed: {e}")


def _run(x, Wq, bq, Wk, bk, Wv, bv, trace=False, trace_kwargs=None):
    _enable_jax_cache()
    if trace:
        _install_ntff_hook_shim()
    from concourse.bass_utils import run_bass_kernel_spmd

    use_bias = bool(np.any(bq) or np.any(bk) or np.any(bv))
    nc = _get_nc(use_bias)
    in_maps = _prep_inputs(x, Wq, bq, Wk, bk, Wv, bv, use_bias)
    res = run_bass_kernel_spmd(
        nc, in_maps, core_ids=list(range(NCORES)),
        trace=trace, **(trace_kwargs or {}),
    )
    out = np.empty((B, S, D), dtype=np.float32)
    for c in range(NCORES):
        b, h = c // 2, c % 2
        out[b, h * HALF:(h + 1) * HALF, :] = res.results[c]["out"]
    return out, res


def kernel(x, Wq, bq, Wk, bk, Wv, bv):
    out, _ = _run(x, Wq, bq, Wk, bk, Wv, bv, trace=False)
    return out


# revision 9
# speedup vs baseline: 1.0396x; 1.0396x over previous
"""AttentionHead kernel for 8 TRN2 NeuronCores (Bass/Tile).

Problem: x[4, 2048, 1024] f32; Wq/Wk/Wv[1024, 1024], bq/bk/bv[1024].
  q = x@Wq+bq ; k = x@Wk+bk ; v = x@Wv+bv
  out = softmax(q k^T / sqrt(1024)) @ v

Sharding: 8 shards = (batch b in 0..3) x (query-half h in 0..1).
Core c = 2*b + h computes output rows [h*1024, (h+1)*1024) of batch b.
Each core's input sequence is ROLLED so its query half occupies tokens
0:1024 (softmax is permutation-invariant over keys, so K/V token order
doesn't matter). Each core does its own K/V projection for the full
2048-token sequence (duplicated across the pair sharing a batch; a
2-rank AllGather measures ~34 GB/s which would cost more than the
~55us of duplicated matmul it saves).

Compute dtype: bf16 operands, f32 PSUM accumulation.
"""

import numpy as np
import ml_dtypes

B = 4
S = 2048
D = 1024
HALF = S // 2  # query rows per core
NCORES = 8
DCH = D // 128  # 8 contraction chunks
BF = ml_dtypes.bfloat16

_cache = {}


def _build(use_bias: bool):
    import concourse.bass as bass
    import concourse.mybir as mybir
    import concourse.tile as tile
    from concourse import bacc
    from concourse.masks import make_identity

    FP32 = mybir.dt.float32
    BF16 = mybir.dt.bfloat16
    AF = mybir.ActivationFunctionType

    nc = bacc.Bacc(
        "TRN2",
        target_bir_lowering=False,
        debug=False,
        enable_asserts=True,
        num_devices=NCORES,
    )

    # Per-core inputs. x rolled: rows 0:1024 are this core's query tokens.
    x_d = nc.dram_tensor("x", [S, D], BF16, kind="ExternalInput").ap()
    # Weights in e-chunk-major layout [e_chunk, d, 128] (Wq pre-scaled 1/32).
    wq_d = nc.dram_tensor("wq", [DCH, D, 128], BF16, kind="ExternalInput").ap()
    wk_d = nc.dram_tensor("wk", [DCH, D, 128], BF16, kind="ExternalInput").ap()
    wv_d = nc.dram_tensor("wv", [D, D], BF16, kind="ExternalInput").ap()
    if use_bias:
        bq_d = nc.dram_tensor("bq", [DCH, 128], FP32, kind="ExternalInput").ap()
        bk_d = nc.dram_tensor("bk", [DCH, 128], FP32, kind="ExternalInput").ap()
        bv_d = nc.dram_tensor("bv", [1, D], FP32, kind="ExternalInput").ap()
    out_d = nc.dram_tensor("out", [HALF, D], FP32, kind="ExternalOutput").ap()

    with tile.TileContext(nc) as tc:
        with (
            tc.tile_pool(name="persist", bufs=1) as persist,
            tc.tile_pool(name="wstream", bufs=2) as wpool,
            tc.tile_pool(name="ppool", bufs=2) as ppool,
            tc.tile_pool(name="stat", bufs=2) as statpool,
            tc.tile_pool(name="opool", bufs=2) as opool,
            tc.tile_pool(name="psA", bufs=1, space="PSUM") as psA,
            tc.tile_pool(name="psB", bufs=2, space="PSUM") as psB,
            tc.tile_pool(name="psO", bufs=1, space="PSUM") as psO,
        ):
            # ---- persistent tiles ----
            ident = persist.tile([128, 128], BF16, tag="ident", name="ident")
            make_identity(nc, ident)

            xt = [persist.tile([128, S], BF16, tag=f"xt{d}", name=f"xt{d}")
                  for d in range(DCH)]
            wv_sb = [persist.tile([128, D], BF16, tag=f"wv{d}", name=f"wv{d}")
                     for d in range(DCH)]
            kT = [persist.tile([128, S], BF16, tag=f"kT{m}", name=f"kT{m}")
                  for m in range(DCH)]
            qT = [persist.tile([128, HALF], BF16, tag=f"qT{m}", name=f"qT{m}")
                  for m in range(DCH)]
            v_sb = [persist.tile([128, D], BF16, tag=f"v{t}", name=f"v{t}")
                    for t in range(S // 128)]

            if use_bias:
                bq_sb = persist.tile([128, DCH], FP32, tag="bq", name="bq_sb")
                bk_sb = persist.tile([128, DCH], FP32, tag="bk", name="bk_sb")
                bv_row = persist.tile([1, D], FP32, tag="bvr", name="bv_row")
                bv_bc = persist.tile([128, D], FP32, tag="bvb", name="bv_bc")
                nc.sync.dma_start(bq_sb, bq_d.rearrange("a b -> b a"))
                nc.sync.dma_start(bk_sb, bk_d.rearrange("a b -> b a"))
                nc.sync.dma_start(bv_row, bv_d)
                nc.gpsimd.partition_broadcast(bv_bc, bv_row)

            # ---- first Q weight chunk before the transposes, so the first
            # matmul can go as soon as the x transposes land ----
            wq_sb0 = wpool.tile([128, DCH, 128], BF16, tag="wq", name="wq_sb0")
            nc.sync.dma_start(wq_sb0, wq_d[0].rearrange("(jd p) e -> p jd e",
                                                        p=128))
            # ---- x transposed into SBUF via DMA-transpose (xbar) ----
            for d in range(DCH):
                nc.sync.dma_start_transpose(xt[d], x_d[:, d * 128:(d + 1) * 128])

            def proj_evac(dst, ps, bias_col):
                if use_bias:
                    nc.scalar.activation(dst, ps, AF.Identity, bias=bias_col)
                else:
                    nc.any.tensor_copy(dst, ps)

            # ---- Q^T projection: qT[m][e, q] = sum_d Wq[d, e] * x[q, d],
            # query tokens are rows 0:1024 of the rolled x ----
            for m in range(DCH):
                if m == 0:
                    wq_sb = wq_sb0
                else:
                    wq_sb = wpool.tile([128, DCH, 128], BF16, tag="wq",
                                       name="wq_sb")
                    nc.sync.dma_start(wq_sb,
                                      wq_d[m].rearrange("(jd p) e -> p jd e",
                                                        p=128))
                for qf in range(HALF // 512):
                    ps = psB.tile([128, 512], FP32, tag="ps_small", name="ps_q")
                    for jd in range(DCH):
                        nc.tensor.matmul(
                            ps,
                            wq_sb[:, jd, :],
                            xt[jd][:, qf * 512:(qf + 1) * 512],
                            start=(jd == 0),
                            stop=(jd == DCH - 1),
                        )
                    proj_evac(qT[m][:, qf * 512:(qf + 1) * 512], ps,
                              bq_sb[:, m:m + 1] if use_bias else None)

            # ---- K^T projection over the full sequence ----
            for m in range(DCH):
                wk_sb = wpool.tile([128, DCH, 128], BF16, tag="wk", name="wk_sb")
                nc.sync.dma_start(wk_sb, wk_d[m].rearrange("(jd p) e -> p jd e",
                                                           p=128))
                for tf in range(S // 512):
                    ps = psB.tile([128, 512], FP32, tag="ps_small", name="ps_k")
                    for jd in range(DCH):
                        nc.tensor.matmul(
                            ps,
                            wk_sb[:, jd, :],
                            xt[jd][:, tf * 512:(tf + 1) * 512],
                            start=(jd == 0),
                            stop=(jd == DCH - 1),
                        )
                    proj_evac(kT[m][:, tf * 512:(tf + 1) * 512], ps,
                              bk_sb[:, m:m + 1] if use_bias else None)

            # ---- V projection (v bias deferred to output: sum_j w_ij = 1) ----
            for d in range(DCH):
                nc.sync.dma_start(wv_sb[d], wv_d[d * 128:(d + 1) * 128, :])
            for t in range(S // 128):
                for ef in range(D // 512):
                    ps = psB.tile([128, 512], FP32, tag="ps_small", name="ps_v")
                    for jd in range(DCH):
                        nc.tensor.matmul(
                            ps,
                            xt[jd][:, t * 128:(t + 1) * 128],
                            wv_sb[jd][:, ef * 512:(ef + 1) * 512],
                            start=(jd == 0),
                            stop=(jd == DCH - 1),
                        )
                    nc.any.tensor_copy(v_sb[t][:, ef * 512:(ef + 1) * 512], ps)

            # ---- attention, 128 query rows per block, 3-stage software
            # pipeline. Emission order inside iteration qb:
            #   S(qb) scores -> out-evac(qb-2) (DVE, runs during S) ->
            #   transposes+PV(qb-1) -> stats(qb) (max/exp/recip).
            # This keeps the in-order DVE queue from blocking PSUM-slot
            # frees and fills the exp latency with block qb-1's PE work. ----
            NQB = HALF // 128
            pend = {}   # qb -> (P, rinv)
            outp = {}   # qb -> psout

            def emit_tail_front(qb):
                P, rinv = pend.pop(qb)
                pT = ppool.tile([128, S], BF16, tag="pT", name="pT")
                for jj in range(2):
                    psT = psB.tile([128, 1024], BF16, tag="ps_small", name="ps_t")
                    for u in range(8):
                        j = jj * 8 + u
                        nc.tensor.transpose(psT[:, u * 128:(u + 1) * 128],
                                            P[:, j * 128:(j + 1) * 128], ident)
                    nc.vector.tensor_copy(pT[:, jj * 1024:(jj + 1) * 1024], psT)

                psout = psO.tile([128, D], FP32, tag="psout", name="psout")
                for ef in range(D // 512):
                    for j in range(S // 128):
                        nc.tensor.matmul(
                            psout[:, ef * 512:(ef + 1) * 512],
                            pT[:, j * 128:(j + 1) * 128],
                            v_sb[j][:, ef * 512:(ef + 1) * 512],
                            start=(j == 0),
                            stop=(j == S // 128 - 1),
                        )
                outp[qb] = (psout, rinv)

            def emit_out_evac(qb):
                psout, rinv = outp.pop(qb)
                osb = opool.tile([128, D], FP32, tag="osb", name="osb")
                nc.vector.tensor_scalar_mul(osb, psout, rinv)
                if use_bias:
                    nc.vector.tensor_add(osb, osb, bv_bc)
                nc.sync.dma_start(out_d[qb * 128:(qb + 1) * 128, :], osb)

            for qb in range(NQB):
                psS = psA.tile([128, S], FP32, tag="psS", name="psS")
                for tf in range(S // 512):
                    for m in range(DCH):
                        nc.tensor.matmul(
                            psS[:, tf * 512:(tf + 1) * 512],
                            qT[m][:, qb * 128:(qb + 1) * 128],
                            kT[m][:, tf * 512:(tf + 1) * 512],
                            start=(m == 0),
                            stop=(m == DCH - 1),
                        )

                if qb >= 2:
                    emit_out_evac(qb - 2)
                if qb >= 1:
                    emit_tail_front(qb - 1)

                negmax = statpool.tile([128, 1], FP32, tag="negmax",
                                       name="negmax")
                nc.vector.reduce_max(negmax, psS, axis=mybir.AxisListType.X,
                                     negate=True)
                P = ppool.tile([128, S], BF16, tag="P", name="P")
                rowsum = statpool.tile([128, 1], FP32, tag="rowsum",
                                       name="rowsum")
                nc.scalar.activation(P, psS, AF.Exp, bias=negmax, scale=1.0,
                                     accum_out=rowsum)
                rinv = statpool.tile([128, 1], FP32, tag="rinv", name="rinv",
                                     bufs=3)
                nc.vector.reciprocal(rinv, rowsum)
                pend[qb] = (P, rinv)

            emit_out_evac(NQB - 2)
            emit_tail_front(NQB - 1)
            emit_out_evac(NQB - 1)

    nc.compile()
    return nc


def _get_nc(use_bias: bool):
    key = ("nc", use_bias)
    if key not in _cache:
        _cache[key] = _build(use_bias)
    return _cache[key]


def _prep_inputs(x, Wq, bq, Wk, bk, Wv, bv, use_bias):
    """Host-side shard + layout prep. Returns in_maps for cores 0..7."""
    scale = 1.0 / np.sqrt(np.float32(D))

    def echunk(w):
        return np.ascontiguousarray(
            w.reshape(D, DCH, 128).transpose(1, 0, 2)).astype(BF)

    wq_r = echunk(np.asarray(Wq, dtype=np.float32) * scale)
    wk_r = echunk(np.asarray(Wk, dtype=np.float32))
    wv_r = np.asarray(Wv, dtype=np.float32).astype(BF)

    if use_bias:
        bq_r = np.ascontiguousarray(
            (np.asarray(bq, np.float32) * scale).reshape(DCH, 128))
        bk_r = np.ascontiguousarray(np.asarray(bk, np.float32).reshape(DCH, 128))
        bv_r = np.ascontiguousarray(np.asarray(bv, np.float32).reshape(1, D))

    x = np.asarray(x, dtype=np.float32)
    in_maps = []
    for c in range(NCORES):
        b, h = c // 2, c % 2
        xb = x[b].astype(BF)
        if h == 1:  # roll: this core's query half first (keys are order-free)
            xb = np.concatenate([xb[HALF:], xb[:HALF]], axis=0)
        xb = np.ascontiguousarray(xb)
        m = {"x": xb, "wq": wq_r, "wk": wk_r, "wv": wv_r}
        if use_bias:
            m.update({"bq": bq_r, "bk": bk_r, "bv": bv_r})
        in_maps.append(m)
    return in_maps


def _enable_jax_cache():
    try:
        import jax

        jax.config.update("jax_compilation_cache_dir", "/tmp/jax_neff_cache")
        jax.config.update("jax_persistent_cache_min_compile_time_secs", 0.0)
        jax.config.update("jax_persistent_cache_min_entry_size_bytes", -1)
    except Exception:
        pass


def _install_ntff_hook_shim():
    """The agent image's antenv lacks axon_hooks; synthesize it from
    trn_boot's ctypes implementation so trace=True can profile."""
    import sys
    import types

    if "antenv.axon_hooks" in sys.modules:
        return
    try:
        import antenv
        from trn_agent_boot.trn_boot import _ntff_profile_via_ctypes

        hook = _ntff_profile_via_ctypes("/opt/axon/libaxon_pjrt.so")
        mod = types.ModuleType("antenv.axon_hooks")
        state = {"h": hook}
        mod.get_axon_ntff_profile_hook = lambda: state["h"]
        mod.set_axon_ntff_profile_hook = lambda h: state.update(h=h)
        antenv.axon_hooks = mod
        sys.modules["antenv.axon_hooks"] = mod
    except Exception as e:
        print(f"ntff hook shim failed: {e}")


def _run(x, Wq, bq, Wk, bk, Wv, bv, trace=False, trace_kwargs=None):
    _enable_jax_cache()
    if trace:
        _install_ntff_hook_shim()
    from concourse.bass_utils import run_bass_kernel_spmd

    use_bias = bool(np.any(bq) or np.any(bk) or np.any(bv))
    nc = _get_nc(use_bias)
    in_maps = _prep_inputs(x, Wq, bq, Wk, bk, Wv, bv, use_bias)
    res = run_bass_kernel_spmd(
        nc, in_maps, core_ids=list(range(NCORES)),
        trace=trace, **(trace_kwargs or {}),
    )
    out = np.empty((B, S, D), dtype=np.float32)
    for c in range(NCORES):
        b, h = c // 2, c % 2
        shard = res.results[c]["out"]
        out[b, h * HALF:(h + 1) * HALF, :] = shard
    return out, res


def kernel(x, Wq, bq, Wk, bk, Wv, bv):
    out, _ = _run(x, Wq, bq, Wk, bk, Wv, bv, trace=False)
    return out


# revision 11
# speedup vs baseline: 1.1839x; 1.1388x over previous
"""AttentionHead kernel for 8 TRN2 NeuronCores (Bass/Tile).

Problem: x[4, 2048, 1024] f32; Wq/Wk/Wv[1024, 1024], bq/bk/bv[1024].
  q = x@Wq+bq ; k = x@Wk+bk ; v = x@Wv+bv
  out = softmax(q k^T / sqrt(1024)) @ v

Sharding: 8 shards = (batch b in 0..3) x (query-half h in 0..1).
Core c = 2*b + h computes output rows [h*1024, (h+1)*1024) of batch b.
Each core's input sequence is ROLLED so its query half occupies tokens
0:1024 (softmax is permutation-invariant over keys, so K/V token order
doesn't matter). Each core does its own K/V projection for the full
2048-token sequence (duplicated across the pair sharing a batch; a
2-rank AllGather measures ~34 GB/s which would cost more than the
~55us of duplicated matmul it saves).

Compute dtype: bf16 operands, f32 PSUM accumulation.
"""

import numpy as np
import ml_dtypes

B = 4
S = 2048
D = 1024
HALF = S // 2  # query rows per core
NCORES = 8
DCH = D // 128  # 8 contraction chunks
BF = ml_dtypes.bfloat16

_cache = {}


def _build(use_bias: bool):
    import concourse.bass as bass
    import concourse.mybir as mybir
    import concourse.tile as tile
    from concourse import bacc
    from concourse.masks import make_identity

    FP32 = mybir.dt.float32
    BF16 = mybir.dt.bfloat16
    AF = mybir.ActivationFunctionType

    nc = bacc.Bacc(
        "TRN2",
        target_bir_lowering=False,
        debug=False,
        enable_asserts=True,
        num_devices=NCORES,
    )

    # Per-core inputs. x rolled: rows 0:1024 are this core's query tokens.
    x_d = nc.dram_tensor("x", [S, D], BF16, kind="ExternalInput").ap()
    # Weights in e-chunk-major layout [e_chunk, d, 128] (Wq pre-scaled 1/32).
    wq_d = nc.dram_tensor("wq", [DCH, D, 128], BF16, kind="ExternalInput").ap()
    wk_d = nc.dram_tensor("wk", [DCH, D, 128], BF16, kind="ExternalInput").ap()
    wv_d = nc.dram_tensor("wv", [D, D], BF16, kind="ExternalInput").ap()
    if use_bias:
        bq_d = nc.dram_tensor("bq", [DCH, 128], FP32, kind="ExternalInput").ap()
        bk_d = nc.dram_tensor("bk", [DCH, 128], FP32, kind="ExternalInput").ap()
        bv_d = nc.dram_tensor("bv", [1, D], FP32, kind="ExternalInput").ap()
    out_d = nc.dram_tensor("out", [HALF, D], FP32, kind="ExternalOutput").ap()

    with tile.TileContext(nc) as tc:
        with (
            tc.tile_pool(name="persist", bufs=1) as persist,
            tc.tile_pool(name="wstream", bufs=2) as wpool,
            tc.tile_pool(name="ppool", bufs=2) as ppool,
            tc.tile_pool(name="stat", bufs=2) as statpool,
            tc.tile_pool(name="opool", bufs=2) as opool,
            tc.tile_pool(name="psA", bufs=1, space="PSUM") as psA,
            tc.tile_pool(name="psB", bufs=2, space="PSUM") as psB,
            tc.tile_pool(name="psO", bufs=1, space="PSUM") as psO,
        ):
            # ---- persistent tiles ----
            ident = persist.tile([128, 128], BF16, tag="ident", name="ident")
            make_identity(nc, ident)

            xt = [persist.tile([128, S], BF16, tag=f"xt{d}", name=f"xt{d}")
                  for d in range(DCH)]
            wv_sb = [persist.tile([128, D], BF16, tag=f"wv{d}", name=f"wv{d}")
                     for d in range(DCH)]
            kT = [persist.tile([128, S], BF16, tag=f"kT{m}", name=f"kT{m}")
                  for m in range(DCH)]
            qT = [persist.tile([128, HALF], BF16, tag=f"qT{m}", name=f"qT{m}")
                  for m in range(DCH)]
            v_sb = [persist.tile([128, D], BF16, tag=f"v{t}", name=f"v{t}")
                    for t in range(S // 128)]

            if use_bias:
                bq_sb = persist.tile([128, DCH], FP32, tag="bq", name="bq_sb")
                bk_sb = persist.tile([128, DCH], FP32, tag="bk", name="bk_sb")
                bv_row = persist.tile([1, D], FP32, tag="bvr", name="bv_row")
                bv_bc = persist.tile([128, D], FP32, tag="bvb", name="bv_bc")
                nc.sync.dma_start(bq_sb, bq_d.rearrange("a b -> b a"))
                nc.sync.dma_start(bk_sb, bk_d.rearrange("a b -> b a"))
                nc.sync.dma_start(bv_row, bv_d)
                nc.gpsimd.partition_broadcast(bv_bc, bv_row)

            # ---- first Q weight chunk before the transposes, so the first
            # matmul can go as soon as the x transposes land ----
            wq_sb0 = wpool.tile([128, DCH, 128], BF16, tag="wq", name="wq_sb0")
            nc.sync.dma_start(wq_sb0, wq_d[0].rearrange("(jd p) e -> p jd e",
                                                        p=128))
            # ---- x transposed into SBUF via DMA-transpose (xbar); query
            # half (rows 0:1024) first so Q^T matmuls can start early ----
            for d in range(DCH):
                nc.sync.dma_start_transpose(
                    xt[d][:, 0:HALF], x_d[0:HALF, d * 128:(d + 1) * 128])
            for d in range(DCH):
                nc.sync.dma_start_transpose(
                    xt[d][:, HALF:S], x_d[HALF:S, d * 128:(d + 1) * 128])

            def proj_evac(dst, ps, bias_col):
                if use_bias:
                    nc.scalar.activation(dst, ps, AF.Identity, bias=bias_col)
                else:
                    nc.any.tensor_copy(dst, ps)

            # ---- Q^T projection: qT[m][e, q] = sum_d Wq[d, e] * x[q, d],
            # query tokens are rows 0:1024 of the rolled x ----
            for m in range(DCH):
                if m == 0:
                    wq_sb = wq_sb0
                else:
                    wq_sb = wpool.tile([128, DCH, 128], BF16, tag="wq",
                                       name="wq_sb")
                    nc.sync.dma_start(wq_sb,
                                      wq_d[m].rearrange("(jd p) e -> p jd e",
                                                        p=128))
                for qf in range(HALF // 512):
                    ps = psB.tile([128, 512], FP32, tag="ps_small", name="ps_q")
                    for jd in range(DCH):
                        nc.tensor.matmul(
                            ps,
                            wq_sb[:, jd, :],
                            xt[jd][:, qf * 512:(qf + 1) * 512],
                            start=(jd == 0),
                            stop=(jd == DCH - 1),
                        )
                    proj_evac(qT[m][:, qf * 512:(qf + 1) * 512], ps,
                              bq_sb[:, m:m + 1] if use_bias else None)

            # ---- K^T projection over the full sequence ----
            for m in range(DCH):
                wk_sb = wpool.tile([128, DCH, 128], BF16, tag="wk", name="wk_sb")
                nc.sync.dma_start(wk_sb, wk_d[m].rearrange("(jd p) e -> p jd e",
                                                           p=128))
                for tf in range(S // 512):
                    ps = psB.tile([128, 512], FP32, tag="ps_small", name="ps_k")
                    for jd in range(DCH):
                        nc.tensor.matmul(
                            ps,
                            wk_sb[:, jd, :],
                            xt[jd][:, tf * 512:(tf + 1) * 512],
                            start=(jd == 0),
                            stop=(jd == DCH - 1),
                        )
                    proj_evac(kT[m][:, tf * 512:(tf + 1) * 512], ps,
                              bk_sb[:, m:m + 1] if use_bias else None)

            # ---- V projection (v bias deferred to output: sum_j w_ij = 1) ----
            for d in range(DCH):
                nc.sync.dma_start(wv_sb[d], wv_d[d * 128:(d + 1) * 128, :])
            for t in range(S // 128):
                for ef in range(D // 512):
                    ps = psB.tile([128, 512], FP32, tag="ps_small", name="ps_v")
                    for jd in range(DCH):
                        nc.tensor.matmul(
                            ps,
                            xt[jd][:, t * 128:(t + 1) * 128],
                            wv_sb[jd][:, ef * 512:(ef + 1) * 512],
                            start=(jd == 0),
                            stop=(jd == DCH - 1),
                        )
                    nc.any.tensor_copy(v_sb[t][:, ef * 512:(ef + 1) * 512], ps)

            # ---- attention, 128 query rows per block, 3-stage software
            # pipeline. Emission order inside iteration qb:
            #   S(qb) scores -> out-evac(qb-2) (DVE, runs during S) ->
            #   transposes+PV(qb-1) -> stats(qb) (max/exp/recip).
            # This keeps the in-order DVE queue from blocking PSUM-slot
            # frees and fills the exp latency with block qb-1's PE work. ----
            NQB = HALF // 128
            pend = {}   # qb -> (P, rinv)
            outp = {}   # qb -> psout

            def emit_tail_front(qb):
                P, rinv = pend.pop(qb)
                pT = ppool.tile([128, S], BF16, tag="pT", name="pT")
                for jj in range(4):
                    psT = psB.tile([128, 512], BF16, tag="ps_small", name="ps_t")
                    for u in range(4):
                        j = jj * 4 + u
                        nc.tensor.transpose(psT[:, u * 128:(u + 1) * 128],
                                            P[:, j * 128:(j + 1) * 128], ident)
                    nc.vector.tensor_copy(pT[:, jj * 512:(jj + 1) * 512], psT)

                psout = psO.tile([128, D], FP32, tag="psout", name="psout")
                for ef in range(D // 512):
                    for j in range(S // 128):
                        nc.tensor.matmul(
                            psout[:, ef * 512:(ef + 1) * 512],
                            pT[:, j * 128:(j + 1) * 128],
                            v_sb[j][:, ef * 512:(ef + 1) * 512],
                            start=(j == 0),
                            stop=(j == S // 128 - 1),
                        )
                outp[qb] = (psout, rinv)

            def emit_out_evac(qb):
                psout, rinv = outp.pop(qb)
                osb = opool.tile([128, D], FP32, tag="osb", name="osb")
                nc.vector.tensor_scalar_mul(osb, psout, rinv)
                if use_bias:
                    nc.vector.tensor_add(osb, osb, bv_bc)
                nc.sync.dma_start(out_d[qb * 128:(qb + 1) * 128, :], osb)

            for qb in range(NQB):
                psS = psA.tile([128, S], FP32, tag="psS", name="psS")
                for tf in range(S // 512):
                    for m in range(DCH):
                        nc.tensor.matmul(
                            psS[:, tf * 512:(tf + 1) * 512],
                            qT[m][:, qb * 128:(qb + 1) * 128],
                            kT[m][:, tf * 512:(tf + 1) * 512],
                            start=(m == 0),
                            stop=(m == DCH - 1),
                        )

                if qb >= 2:
                    emit_out_evac(qb - 2)
                if qb >= 1:
                    emit_tail_front(qb - 1)

                negmax = statpool.tile([128, 1], FP32, tag="negmax",
                                       name="negmax")
                nc.vector.reduce_max(negmax, psS, axis=mybir.AxisListType.X,
                                     negate=True)
                P = ppool.tile([128, S], BF16, tag="P", name="P")
                rowsum = statpool.tile([128, 1], FP32, tag="rowsum",
                                       name="rowsum")
                nc.scalar.activation(P, psS, AF.Exp, bias=negmax, scale=1.0,
                                     accum_out=rowsum)
                rinv = statpool.tile([128, 1], FP32, tag="rinv", name="rinv",
                                     bufs=3)
                nc.vector.reciprocal(rinv, rowsum)
                pend[qb] = (P, rinv)

            emit_out_evac(NQB - 2)
            emit_tail_front(NQB - 1)
            emit_out_evac(NQB - 1)

    nc.compile()
    return nc


def _get_nc(use_bias: bool):
    key = ("nc", use_bias)
    if key not in _cache:
        _cache[key] = _build(use_bias)
    return _cache[key]


def _prep_inputs(x, Wq, bq, Wk, bk, Wv, bv, use_bias):
    """Host-side shard + layout prep. Returns in_maps for cores 0..7."""
    scale = 1.0 / np.sqrt(np.float32(D))

    def echunk(w):
        return np.ascontiguousarray(
            w.reshape(D, DCH, 128).transpose(1, 0, 2)).astype(BF)

    wq_r = echunk(np.asarray(Wq, dtype=np.float32) * scale)
    wk_r = echunk(np.asarray(Wk, dtype=np.float32))
    wv_r = np.asarray(Wv, dtype=np.float32).astype(BF)

    if use_bias:
        bq_r = np.ascontiguousarray(
            (np.asarray(bq, np.float32) * scale).reshape(DCH, 128))
        bk_r = np.ascontiguousarray(np.asarray(bk, np.float32).reshape(DCH, 128))
        bv_r = np.ascontiguousarray(np.asarray(bv, np.float32).reshape(1, D))

    x = np.asarray(x, dtype=np.float32)
    in_maps = []
    for c in range(NCORES):
        b, h = c // 2, c % 2
        xb = x[b].astype(BF)
        if h == 1:  # roll: this core's query half first (keys are order-free)
            xb = np.concatenate([xb[HALF:], xb[:HALF]], axis=0)
        xb = np.ascontiguousarray(xb)
        m = {"x": xb, "wq": wq_r, "wk": wk_r, "wv": wv_r}
        if use_bias:
            m.update({"bq": bq_r, "bk": bk_r, "bv": bv_r})
        in_maps.append(m)
    return in_maps


def _enable_jax_cache():
    try:
        import jax

        jax.config.update("jax_compilation_cache_dir", "/tmp/jax_neff_cache")
        jax.config.update("jax_persistent_cache_min_compile_time_secs", 0.0)
        jax.config.update("jax_persistent_cache_min_entry_size_bytes", -1)
    except Exception:
        pass


def _install_ntff_hook_shim():
    """The agent image's antenv lacks axon_hooks; synthesize it from
    trn_boot's ctypes implementation so trace=True can profile."""
    import sys
    import types

    if "antenv.axon_hooks" in sys.modules:
        return
    try:
        import antenv
        from trn_agent_boot.trn_boot import _ntff_profile_via_ctypes

        hook = _ntff_profile_via_ctypes("/opt/axon/libaxon_pjrt.so")
        mod = types.ModuleType("antenv.axon_hooks")
        state = {"h": hook}
        mod.get_axon_ntff_profile_hook = lambda: state["h"]
        mod.set_axon_ntff_profile_hook = lambda h: state.update(h=h)
        antenv.axon_hooks = mod
        sys.modules["antenv.axon_hooks"] = mod
    except Exception as e:
        print(f"ntff hook shim failed: {e}")


def _run(x, Wq, bq, Wk, bk, Wv, bv, trace=False, trace_kwargs=None):
    _enable_jax_cache()
    if trace:
        _install_ntff_hook_shim()
    from concourse.bass_utils import run_bass_kernel_spmd

    use_bias = bool(np.any(bq) or np.any(bk) or np.any(bv))
    nc = _get_nc(use_bias)
    in_maps = _prep_inputs(x, Wq, bq, Wk, bk, Wv, bv, use_bias)
    res = run_bass_kernel_spmd(
        nc, in_maps, core_ids=list(range(NCORES)),
        trace=trace, **(trace_kwargs or {}),
    )
    out = np.empty((B, S, D), dtype=np.float32)
    for c in range(NCORES):
        b, h = c // 2, c % 2
        shard = res.results[c]["out"]
        out[b, h * HALF:(h + 1) * HALF, :] = shard
    return out, res


def kernel(x, Wq, bq, Wk, bk, Wv, bv):
    out, _ = _run(x, Wq, bq, Wk, bk, Wv, bv, trace=False)
    return out


# revision 12
# speedup vs baseline: 1.2409x; 1.0481x over previous
"""AttentionHead kernel for 8 TRN2 NeuronCores (Bass/Tile).

Problem: x[4, 2048, 1024] f32; Wq/Wk/Wv[1024, 1024], bq/bk/bv[1024].
  q = x@Wq+bq ; k = x@Wk+bk ; v = x@Wv+bv
  out = softmax(q k^T / sqrt(1024)) @ v

Sharding: 8 shards = (batch b in 0..3) x (query-half h in 0..1).
Core c = 2*b + h computes output rows [h*1024, (h+1)*1024) of batch b.
Each core's input sequence is ROLLED so its query half occupies tokens
0:1024 (softmax is permutation-invariant over keys, so K/V token order
doesn't matter). Each core does its own K/V projection for the full
2048-token sequence (duplicated across the pair sharing a batch; a
2-rank AllGather measures ~34 GB/s which would cost more than the
~55us of duplicated matmul it saves).

Compute dtype: bf16 operands, f32 PSUM accumulation.
"""

import numpy as np
import ml_dtypes

B = 4
S = 2048
D = 1024
HALF = S // 2  # query rows per core
NCORES = 8
DCH = D // 128  # 8 contraction chunks
BF = ml_dtypes.bfloat16

_cache = {}


def _build(use_bias: bool):
    import concourse.bass as bass
    import concourse.mybir as mybir
    import concourse.tile as tile
    from concourse import bacc
    from concourse.masks import make_identity

    FP32 = mybir.dt.float32
    BF16 = mybir.dt.bfloat16
    AF = mybir.ActivationFunctionType

    nc = bacc.Bacc(
        "TRN2",
        target_bir_lowering=False,
        debug=False,
        enable_asserts=True,
        num_devices=NCORES,
    )

    # Per-core inputs. x rolled: rows 0:1024 are this core's query tokens.
    x_d = nc.dram_tensor("x", [S, D], BF16, kind="ExternalInput").ap()
    # Weights in e-chunk-major layout [e_chunk, d, 128] (Wq pre-scaled 1/32).
    wq_d = nc.dram_tensor("wq", [DCH, D, 128], BF16, kind="ExternalInput").ap()
    wk_d = nc.dram_tensor("wk", [DCH, D, 128], BF16, kind="ExternalInput").ap()
    wv_d = nc.dram_tensor("wv", [D, D], BF16, kind="ExternalInput").ap()
    if use_bias:
        bq_d = nc.dram_tensor("bq", [DCH, 128], FP32, kind="ExternalInput").ap()
        bk_d = nc.dram_tensor("bk", [DCH, 128], FP32, kind="ExternalInput").ap()
        bv_d = nc.dram_tensor("bv", [1, D], FP32, kind="ExternalInput").ap()
    out_d = nc.dram_tensor("out", [HALF, D], FP32, kind="ExternalOutput").ap()

    with tile.TileContext(nc) as tc:
        with (
            tc.tile_pool(name="persist", bufs=1) as persist,
            tc.tile_pool(name="wstream", bufs=2) as wpool,
            tc.tile_pool(name="ppool", bufs=2) as ppool,
            tc.tile_pool(name="stat", bufs=2) as statpool,
            tc.tile_pool(name="opool", bufs=2) as opool,
            tc.tile_pool(name="psA", bufs=1, space="PSUM") as psA,
            tc.tile_pool(name="psB", bufs=2, space="PSUM") as psB,
            tc.tile_pool(name="psO", bufs=1, space="PSUM") as psO,
        ):
            # ---- persistent tiles ----
            ident = persist.tile([128, 128], BF16, tag="ident", name="ident")
            make_identity(nc, ident)

            xt = [persist.tile([128, S], BF16, tag=f"xt{d}", name=f"xt{d}")
                  for d in range(DCH)]
            wv_sb = [persist.tile([128, D], BF16, tag=f"wv{d}", name=f"wv{d}")
                     for d in range(DCH)]
            kT = [persist.tile([128, S], BF16, tag=f"kT{m}", name=f"kT{m}")
                  for m in range(DCH)]
            qT = [persist.tile([128, HALF], BF16, tag=f"qT{m}", name=f"qT{m}")
                  for m in range(DCH)]
            v_sb = [persist.tile([128, D], BF16, tag=f"v{t}", name=f"v{t}")
                    for t in range(S // 128)]

            if use_bias:
                bq_sb = persist.tile([128, DCH], FP32, tag="bq", name="bq_sb")
                bk_sb = persist.tile([128, DCH], FP32, tag="bk", name="bk_sb")
                bv_row = persist.tile([1, D], FP32, tag="bvr", name="bv_row")
                bv_bc = persist.tile([128, D], FP32, tag="bvb", name="bv_bc")
                nc.sync.dma_start(bq_sb, bq_d.rearrange("a b -> b a"))
                nc.sync.dma_start(bk_sb, bk_d.rearrange("a b -> b a"))
                nc.sync.dma_start(bv_row, bv_d)
                nc.gpsimd.partition_broadcast(bv_bc, bv_row)

            # ---- first Q weight chunk before the transposes, so the first
            # matmul can go as soon as the x transposes land ----
            wq_sb0 = wpool.tile([128, DCH, 128], BF16, tag="wq", name="wq_sb0")
            nc.sync.dma_start(wq_sb0, wq_d[0].rearrange("(jd p) e -> p jd e",
                                                        p=128))
            # ---- x transposed into SBUF via DMA-transpose (xbar); query
            # half (rows 0:1024) first so Q^T matmuls can start early; the
            # second half is emitted after the Q^T weight DMAs so those
            # don't queue behind it on the sync HWDGE FIFO ----
            for d in range(DCH):
                nc.sync.dma_start_transpose(
                    xt[d][:, 0:HALF], x_d[0:HALF, d * 128:(d + 1) * 128])

            # exp activation-table prefetch (hides the ~2.7us table load)
            dummy = persist.tile([128, 1], FP32, tag="dummy", name="dummy")
            nc.gpsimd.memset(dummy, 0.0)
            nc.scalar.activation(dummy, dummy, AF.Exp)

            def proj_evac(dst, ps, bias_col):
                if use_bias:
                    nc.scalar.activation(dst, ps, AF.Identity, bias=bias_col)
                else:
                    nc.any.tensor_copy(dst, ps)

            # ---- Q^T projection: qT[m][e, q] = sum_d Wq[d, e] * x[q, d],
            # query tokens are rows 0:1024 of the rolled x ----
            for m in range(DCH):
                if m == 0:
                    wq_sb = wq_sb0
                else:
                    wq_sb = wpool.tile([128, DCH, 128], BF16, tag="wq",
                                       name="wq_sb")
                    nc.sync.dma_start(wq_sb,
                                      wq_d[m].rearrange("(jd p) e -> p jd e",
                                                        p=128))
                for qf in range(HALF // 512):
                    ps = psB.tile([128, 512], FP32, tag="ps_small", name="ps_q")
                    for jd in range(DCH):
                        nc.tensor.matmul(
                            ps,
                            wq_sb[:, jd, :],
                            xt[jd][:, qf * 512:(qf + 1) * 512],
                            start=(jd == 0),
                            stop=(jd == DCH - 1),
                        )
                    proj_evac(qT[m][:, qf * 512:(qf + 1) * 512], ps,
                              bq_sb[:, m:m + 1] if use_bias else None)

            # ---- K^T projection over the full sequence ----
            wk_pre = []
            for m in range(2):
                wk_sb = wpool.tile([128, DCH, 128], BF16, tag="wk", name="wk_sb")
                nc.sync.dma_start(wk_sb, wk_d[m].rearrange("(jd p) e -> p jd e",
                                                           p=128))
                wk_pre.append(wk_sb)
            for d in range(DCH):
                nc.sync.dma_start_transpose(
                    xt[d][:, HALF:S], x_d[HALF:S, d * 128:(d + 1) * 128])
            for m in range(DCH):
                if m < 2:
                    wk_sb = wk_pre[m]
                else:
                    wk_sb = wpool.tile([128, DCH, 128], BF16, tag="wk",
                                       name="wk_sb")
                    nc.sync.dma_start(wk_sb,
                                      wk_d[m].rearrange("(jd p) e -> p jd e",
                                                        p=128))
                for tf in range(S // 512):
                    ps = psB.tile([128, 512], FP32, tag="ps_small", name="ps_k")
                    for jd in range(DCH):
                        nc.tensor.matmul(
                            ps,
                            wk_sb[:, jd, :],
                            xt[jd][:, tf * 512:(tf + 1) * 512],
                            start=(jd == 0),
                            stop=(jd == DCH - 1),
                        )
                    proj_evac(kT[m][:, tf * 512:(tf + 1) * 512], ps,
                              bk_sb[:, m:m + 1] if use_bias else None)

            # ---- V projection (v bias deferred to output: sum_j w_ij = 1) ----
            for d in range(DCH):
                nc.sync.dma_start(wv_sb[d], wv_d[d * 128:(d + 1) * 128, :])
            for t in range(S // 128):
                for ef in range(D // 512):
                    ps = psB.tile([128, 512], FP32, tag="ps_small", name="ps_v")
                    for jd in range(DCH):
                        nc.tensor.matmul(
                            ps,
                            xt[jd][:, t * 128:(t + 1) * 128],
                            wv_sb[jd][:, ef * 512:(ef + 1) * 512],
                            start=(jd == 0),
                            stop=(jd == DCH - 1),
                        )
                    nc.any.tensor_copy(v_sb[t][:, ef * 512:(ef + 1) * 512], ps)

            # ---- attention, 128 query rows per block, 3-stage software
            # pipeline. Emission order inside iteration qb:
            #   S(qb) scores -> out-evac(qb-2) (DVE, runs during S) ->
            #   transposes+PV(qb-1) -> stats(qb) (max/exp/recip).
            # This keeps the in-order DVE queue from blocking PSUM-slot
            # frees and fills the exp latency with block qb-1's PE work. ----
            NQB = HALF // 128
            pend = {}   # qb -> (P, rinv)
            outp = {}   # qb -> psout

            def emit_tail_front(qb):
                P, rinv = pend.pop(qb)
                pT = ppool.tile([128, S], BF16, tag="pT", name="pT")
                for jj in range(4):
                    psT = psB.tile([128, 512], BF16, tag="ps_small", name="ps_t")
                    for u in range(4):
                        j = jj * 4 + u
                        nc.tensor.transpose(psT[:, u * 128:(u + 1) * 128],
                                            P[:, j * 128:(j + 1) * 128], ident)
                    nc.scalar.copy(pT[:, jj * 512:(jj + 1) * 512], psT)

                psout = psO.tile([128, D], FP32, tag="psout", name="psout")
                for ef in range(D // 512):
                    for j in range(S // 128):
                        nc.tensor.matmul(
                            psout[:, ef * 512:(ef + 1) * 512],
                            pT[:, j * 128:(j + 1) * 128],
                            v_sb[j][:, ef * 512:(ef + 1) * 512],
                            start=(j == 0),
                            stop=(j == S // 128 - 1),
                        )
                outp[qb] = (psout, rinv)

            def emit_out_evac(qb):
                psout, rinv = outp.pop(qb)
                osb = opool.tile([128, D], FP32, tag="osb", name="osb")
                nc.vector.tensor_scalar_mul(osb, psout, rinv)
                if use_bias:
                    nc.vector.tensor_add(osb, osb, bv_bc)
                nc.sync.dma_start(out_d[qb * 128:(qb + 1) * 128, :], osb)

            for qb in range(NQB):
                psS = psA.tile([128, S], FP32, tag="psS", name="psS")
                for tf in range(S // 512):
                    for m in range(DCH):
                        nc.tensor.matmul(
                            psS[:, tf * 512:(tf + 1) * 512],
                            qT[m][:, qb * 128:(qb + 1) * 128],
                            kT[m][:, tf * 512:(tf + 1) * 512],
                            start=(m == 0),
                            stop=(m == DCH - 1),
                        )

                if qb >= 2:
                    emit_out_evac(qb - 2)
                if qb >= 1:
                    emit_tail_front(qb - 1)

                negmax = statpool.tile([128, 1], FP32, tag="negmax",
                                       name="negmax")
                nc.vector.reduce_max(negmax, psS, axis=mybir.AxisListType.X,
                                     negate=True)
                P = ppool.tile([128, S], BF16, tag="P", name="P")
                rowsum = statpool.tile([128, 1], FP32, tag="rowsum",
                                       name="rowsum")
                nc.scalar.activation(P, psS, AF.Exp, bias=negmax, scale=1.0,
                                     accum_out=rowsum)
                rinv = statpool.tile([128, 1], FP32, tag="rinv", name="rinv",
                                     bufs=3)
                nc.vector.reciprocal(rinv, rowsum)
                pend[qb] = (P, rinv)

            emit_out_evac(NQB - 2)
            emit_tail_front(NQB - 1)
            emit_out_evac(NQB - 1)

    nc.compile()
    return nc


def _get_nc(use_bias: bool):
    key = ("nc", use_bias)
    if key not in _cache:
        _cache[key] = _build(use_bias)
    return _cache[key]


def _prep_inputs(x, Wq, bq, Wk, bk, Wv, bv, use_bias):
    """Host-side shard + layout prep. Returns in_maps for cores 0..7."""
    scale = 1.0 / np.sqrt(np.float32(D))

    def echunk(w):
        return np.ascontiguousarray(
            w.reshape(D, DCH, 128).transpose(1, 0, 2)).astype(BF)

    wq_r = echunk(np.asarray(Wq, dtype=np.float32) * scale)
    wk_r = echunk(np.asarray(Wk, dtype=np.float32))
    wv_r = np.asarray(Wv, dtype=np.float32).astype(BF)

    if use_bias:
        bq_r = np.ascontiguousarray(
            (np.asarray(bq, np.float32) * scale).reshape(DCH, 128))
        bk_r = np.ascontiguousarray(np.asarray(bk, np.float32).reshape(DCH, 128))
        bv_r = np.ascontiguousarray(np.asarray(bv, np.float32).reshape(1, D))

    x = np.asarray(x, dtype=np.float32)
    in_maps = []
    for c in range(NCORES):
        b, h = c // 2, c % 2
        xb = x[b].astype(BF)
        if h == 1:  # roll: this core's query half first (keys are order-free)
            xb = np.concatenate([xb[HALF:], xb[:HALF]], axis=0)
        xb = np.ascontiguousarray(xb)
        m = {"x": xb, "wq": wq_r, "wk": wk_r, "wv": wv_r}
        if use_bias:
            m.update({"bq": bq_r, "bk": bk_r, "bv": bv_r})
        in_maps.append(m)
    return in_maps


def _enable_jax_cache():
    try:
        import jax

        jax.config.update("jax_compilation_cache_dir", "/tmp/jax_neff_cache")
        jax.config.update("jax_persistent_cache_min_compile_time_secs", 0.0)
        jax.config.update("jax_persistent_cache_min_entry_size_bytes", -1)
    except Exception:
        pass


def _install_ntff_hook_shim():
    """The agent image's antenv lacks axon_hooks; synthesize it from
    trn_boot's ctypes implementation so trace=True can profile."""
    import sys
    import types

    if "antenv.axon_hooks" in sys.modules:
        return
    try:
        import antenv
        from trn_agent_boot.trn_boot import _ntff_profile_via_ctypes

        hook = _ntff_profile_via_ctypes("/opt/axon/libaxon_pjrt.so")
        mod = types.ModuleType("antenv.axon_hooks")
        state = {"h": hook}
        mod.get_axon_ntff_profile_hook = lambda: state["h"]
        mod.set_axon_ntff_profile_hook = lambda h: state.update(h=h)
        antenv.axon_hooks = mod
        sys.modules["antenv.axon_hooks"] = mod
    except Exception as e:
        print(f"ntff hook shim failed: {e}")


def _run(x, Wq, bq, Wk, bk, Wv, bv, trace=False, trace_kwargs=None):
    _enable_jax_cache()
    if trace:
        _install_ntff_hook_shim()
    from concourse.bass_utils import run_bass_kernel_spmd

    use_bias = bool(np.any(bq) or np.any(bk) or np.any(bv))
    nc = _get_nc(use_bias)
    in_maps = _prep_inputs(x, Wq, bq, Wk, bk, Wv, bv, use_bias)
    res = run_bass_kernel_spmd(
        nc, in_maps, core_ids=list(range(NCORES)),
        trace=trace, **(trace_kwargs or {}),
    )
    out = np.empty((B, S, D), dtype=np.float32)
    for c in range(NCORES):
        b, h = c // 2, c % 2
        shard = res.results[c]["out"]
        out[b, h * HALF:(h + 1) * HALF, :] = shard
    return out, res


def kernel(x, Wq, bq, Wk, bk, Wv, bv):
    out, _ = _run(x, Wq, bq, Wk, bk, Wv, bv, trace=False)
    return out
